# Initial kernel scaffold
#
"""Stack-style neural memory kernel for Trainium2 (8 NeuronCores, SPMD).

Reference semantics: at step t, push (d1,v1),(d2,v2); read up to total
strength u_t from the top of the stack; pop strength u_t.  The read
summary is linear in the pushed values:

    out[t,b,:] = sum_j W[t,j,b] * V[j,b,:]      (j = slot index, 2T slots)

where the weights W depend only on the (T,B,1)-sized strength tensors
(u,d1,d2).  W is computed on host (tiny sequential bookkeeping, ~4M
scalar ops); the device does the memory-heavy part: per batch element a
(T x 2T) @ (2T x R) matmul, batch-parallel across 8 cores.
"""

import numpy as np

T, B, R = 128, 128, 512
NSLOTS = 2 * T
N_CORES = 8
BSH = B // N_CORES  # batch shard per core

_NC_CACHE = {}


def _compute_weights(u, d1, d2):
    """W[t, j, b]: read weight of slot j at step t (float32 (T, 2T, B))."""
    uu = u[:, :, 0]
    S = np.zeros((NSLOTS, B), np.float32)
    W = np.empty((T, NSLOTS, B), np.float32)
    for t in range(T):
        S[2 * t] = d1[t, :, 0]
        S[2 * t + 1] = d2[t, :, 0]
        # strength of slots above j (stack top = highest index first)
        c = np.cumsum(S[::-1], axis=0)[::-1]
        cum = c - S
        avail = uu[t][None, :] - cum
        # reference takes a GLOBAL max over the batch for the read scale
        scal = avail.max(axis=1)
        Wt = np.minimum(S, scal[:, None])
        Wt[2 * t + 2:] = 0.0  # slots not yet pushed hold V=0 in the reference
        W[t] = Wt
        # pop u_t: elementwise depletion, same slot order, same cum
        S -= np.minimum(S, np.maximum(0.0, avail))
    return W


def _build_nc():
    from concourse import bacc, tile, mybir

    DT = mybir.dt.float32
    nc = bacc.Bacc(None)
    wt = nc.declare_dram_parameter("wt", [2, 128, BSH * 128], DT, isOutput=False)
    v = nc.declare_dram_parameter("v", [2, 128, BSH * 512], DT, isOutput=False)
    o = nc.declare_dram_parameter("o", [128, BSH * 512], DT, isOutput=True)

    with tile.TileContext(nc) as tc:
        with (
            tc.tile_pool(name="sb", bufs=1) as sb,
            tc.tile_pool(name="ps", bufs=8, space="PSUM") as ps,
        ):
            wt_t = sb.tile([128, 2, BSH * 128], DT, tag="wt")
            v_t = sb.tile([128, 2, BSH * 512], DT, tag="v")
            out_t = sb.tile([128, BSH * 512], DT, tag="out")
            for kc in range(2):
                nc.sync.dma_start(wt_t[:, kc], wt[kc])
                nc.sync.dma_start(v_t[:, kc], v[kc])
            for b in range(BSH):
                acc = ps.tile([128, 512], DT)
                nc.tensor.matmul(
                    acc[:],
                    wt_t[:, 0, b * 128:(b + 1) * 128],
                    v_t[:, 0, b * 512:(b + 1) * 512],
                    start=True,
                    stop=False,
                )
                nc.tensor.matmul(
                    acc[:],
                    wt_t[:, 1, b * 128:(b + 1) * 128],
                    v_t[:, 1, b * 512:(b + 1) * 512],
                    start=False,
                    stop=True,
                )
                nc.vector.tensor_copy(out_t[:, b * 512:(b + 1) * 512], acc[:])
                nc.sync.dma_start(o[:, b * 512:(b + 1) * 512],
                                  out_t[:, b * 512:(b + 1) * 512])
    nc.compile()
    return nc


def kernel(u, d1, d2, v1, v2):
    from concourse.bass_utils import run_bass_kernel_spmd

    u = np.ascontiguousarray(np.asarray(u, np.float32))
    d1 = np.ascontiguousarray(np.asarray(d1, np.float32))
    d2 = np.ascontiguousarray(np.asarray(d2, np.float32))
    v1 = np.ascontiguousarray(np.asarray(v1, np.float32))
    v2 = np.ascontiguousarray(np.asarray(v2, np.float32))

    W = _compute_weights(u, d1, d2)  # (T, 2T, B)

    Vfull = np.empty((NSLOTS, B, R), np.float32)
    Vfull[0::2] = v1
    Vfull[1::2] = v2

    in_maps = []
    for c in range(N_CORES):
        gb = slice(c * BSH, (c + 1) * BSH)
        # lhsT layout: wt[kc, k, b*128 + m] = W[m, kc*128 + k, b]
        wtc = np.ascontiguousarray(
            W[:, :, gb].reshape(T, 2, 128, BSH).transpose(1, 2, 3, 0)
        ).reshape(2, 128, BSH * 128)
        # moving layout: v[kc, k, b*512 + r] = Vfull[kc*128 + k, b, r]
        vc = np.ascontiguousarray(Vfull[:, gb, :]).reshape(2, 128, BSH * 512)
        in_maps.append({"wt": wtc, "v": vc})

    if "nc" not in _NC_CACHE:
        _NC_CACHE["nc"] = _build_nc()
    res = run_bass_kernel_spmd(_NC_CACHE["nc"], in_maps, list(range(N_CORES)))

    out = np.concatenate(
        [res.results[c]["o"].reshape(T, BSH, R) for c in range(N_CORES)], axis=1
    )
    return np.ascontiguousarray(out)


if __name__ == "__main__":
    rng = np.random.default_rng(0)
    ins = {
        "u": rng.random((T, B, 1), np.float32),
        "d1": rng.random((T, B, 1), np.float32),
        "d2": rng.random((T, B, 1), np.float32),
        "v1": rng.standard_normal((T, B, R), dtype=np.float32),
        "v2": rng.standard_normal((T, B, R), dtype=np.float32),
    }
    out = kernel(**ins)
    print(out.shape, out.dtype)


# revision 4
# speedup vs baseline: 2.8602x; 2.8602x over previous
"""Stack-style neural memory kernel for Trainium2 (8 NeuronCores, SPMD).

Reference semantics: at step t, push (d1,v1),(d2,v2); read up to total
strength u_t from the top of the stack; pop strength u_t.  The read
summary is linear in the pushed values:

    out[t,b,:] = sum_j W[t,j,b] * V[j,b,:]      (j = slot index, 2T slots)

where the weights W depend only on the (T,B,1)-sized strength tensors
(u,d1,d2).  W is computed on host (tiny sequential bookkeeping, ~4M
scalar ops; it also needs a global max over the whole batch, which would
otherwise force cross-core communication).  The device does the
memory-heavy part: per batch element a (T x 2T) @ (2T x R) matmul,
batch-parallel across 8 cores with no communication.

Per-core data: V shard 8MB + W shard 2MB in, 4MB out  ->  memory-bound.
"""

import numpy as np

T, B, R = 128, 128, 512
NSLOTS = 2 * T
N_CORES = 8
BSH = B // N_CORES  # batch shard per core
GRP = 4             # batches per DMA group (1MB v-loads / 1MB stores)
NGRP = BSH // GRP

_NC_CACHE = {}


def _compute_weights(u, d1, d2):
    """W[t, j, b]: read weight of slot j at step t (float32 (T, 2T, B))."""
    uu = u[:, :, 0]
    S = np.zeros((NSLOTS, B), np.float32)
    W = np.empty((T, NSLOTS, B), np.float32)
    for t in range(T):
        S[2 * t] = d1[t, :, 0]
        S[2 * t + 1] = d2[t, :, 0]
        # strength of slots above j (stack top = highest index first)
        c = np.cumsum(S[::-1], axis=0)[::-1]
        cum = c - S
        avail = uu[t][None, :] - cum
        # reference takes a GLOBAL max over the batch for the read scale
        scal = avail.max(axis=1)
        Wt = np.minimum(S, scal[:, None])
        Wt[2 * t + 2:] = 0.0  # slots not yet pushed hold V=0 in the reference
        W[t] = Wt
        # pop u_t: elementwise depletion, same slot order, same cum
        S -= np.minimum(S, np.maximum(0.0, avail))
    return W


def _build_nc(reps=1, loop_n=1):
    import contextlib

    from concourse import bacc, tile, mybir

    DT = mybir.dt.float32
    nc = bacc.Bacc(None)
    wt = nc.declare_dram_parameter("wt", [2, 128, BSH * 128], DT, isOutput=False)
    # v grouped by batch: [b, kc, k, r] so a 4-batch chunk is 1MB contiguous
    v = nc.declare_dram_parameter("v", [BSH, 2, 128, 512], DT, isOutput=False)
    # output grouped by batch: o[b, t, r]
    o = nc.declare_dram_parameter("o", [BSH, 128, 512], DT, isOutput=True)

    with tile.TileContext(nc) as tc:
        with (
            tc.tile_pool(name="wtp", bufs=1) as wtp,
            tc.tile_pool(name="vp", bufs=3) as vp,
            tc.tile_pool(name="op", bufs=3) as op,
            tc.tile_pool(name="ps", bufs=8, space="PSUM") as ps,
        ):
            loop_cm = (
                tc.For_i(0, loop_n, 1) if loop_n > 1 else contextlib.nullcontext()
            )
            with loop_cm:
                for rep in range(reps):
                    wt_t = wtp.tile([128, 2, BSH * 128], DT, tag="wt")
                    nc.sync.dma_start(
                        wt_t[:], wt[:, :, :].rearrange("c k n -> k c n")
                    )
                    for g in range(NGRP):
                        # [k, b, kc, r] view of the 1MB chunk v[g*GRP:(g+1)*GRP]
                        v_t = vp.tile([128, GRP, 2, 512], DT, tag="v")
                        nc.sync.dma_start(
                            v_t[:],
                            v[g * GRP:(g + 1) * GRP].rearrange("b c k r -> k b c r"),
                        )
                        out_t = op.tile([128, GRP, 512], DT, tag="out")
                        for bi in range(GRP):
                            b = g * GRP + bi
                            acc = ps.tile([128, 512], DT)
                            nc.tensor.matmul(
                                acc[:],
                                wt_t[:, 0, b * 128:(b + 1) * 128],
                                v_t[:, bi, 0],
                                start=True,
                                stop=False,
                            )
                            nc.tensor.matmul(
                                acc[:],
                                wt_t[:, 1, b * 128:(b + 1) * 128],
                                v_t[:, bi, 1],
                                start=False,
                                stop=True,
                            )
                            nc.vector.tensor_copy(out_t[:, bi], acc[:])
                        nc.sync.dma_start(
                            o[g * GRP:(g + 1) * GRP].rearrange("b t r -> t b r"),
                            out_t[:],
                        )
    nc.compile()
    return nc


def _make_in_maps(u, d1, d2, v1, v2):
    W = _compute_weights(u, d1, d2)  # (T, 2T, B)

    Vfull = np.empty((NSLOTS, B, R), np.float32)
    Vfull[0::2] = v1
    Vfull[1::2] = v2

    in_maps = []
    for c in range(N_CORES):
        gb = slice(c * BSH, (c + 1) * BSH)
        # lhsT layout: wt[kc, k, b*128 + m] = W[m, kc*128 + k, b]
        wtc = np.ascontiguousarray(
            W[:, :, gb].reshape(T, 2, 128, BSH).transpose(1, 2, 3, 0)
        ).reshape(2, 128, BSH * 128)
        # moving layout: v[b, kc, k, r] = Vfull[kc*128 + k, b, r]
        vc = np.ascontiguousarray(
            Vfull[:, gb, :].reshape(2, 128, BSH, 512).transpose(2, 0, 1, 3)
        )
        in_maps.append({"wt": wtc, "v": vc})
    return in_maps


def kernel(u, d1, d2, v1, v2):
    from concourse.bass_utils import run_bass_kernel_spmd

    u = np.ascontiguousarray(np.asarray(u, np.float32))
    d1 = np.ascontiguousarray(np.asarray(d1, np.float32))
    d2 = np.ascontiguousarray(np.asarray(d2, np.float32))
    v1 = np.ascontiguousarray(np.asarray(v1, np.float32))
    v2 = np.ascontiguousarray(np.asarray(v2, np.float32))

    in_maps = _make_in_maps(u, d1, d2, v1, v2)

    if "nc" not in _NC_CACHE:
        _NC_CACHE["nc"] = _build_nc()
    res = run_bass_kernel_spmd(_NC_CACHE["nc"], in_maps, list(range(N_CORES)))

    # o[b, t, r] per core  ->  out[t, b_global, r]
    out = np.concatenate(
        [res.results[c]["o"].transpose(1, 0, 2) for c in range(N_CORES)], axis=1
    )
    return np.ascontiguousarray(out)


if __name__ == "__main__":
    rng = np.random.default_rng(0)
    ins = {
        "u": rng.random((T, B, 1), dtype=np.float32),
        "d1": rng.random((T, B, 1), dtype=np.float32),
        "d2": rng.random((T, B, 1), dtype=np.float32),
        "v1": rng.standard_normal((T, B, R), dtype=np.float32),
        "v2": rng.standard_normal((T, B, R), dtype=np.float32),
    }
    out = kernel(**ins)
    print(out.shape, out.dtype)
